# revision 1
# baseline (speedup 1.0000x reference)
"""Multi-head cross-attention Trainium2 kernel (8-core SPMD, data-parallel).

Shards (batch=4) x (seq halves) across 8 NeuronCores; each core runs the
full q/kv/attention/out-proj pipeline for its 2048 query rows in bf16 with
fp32 PSUM accumulation.

Key tricks:
  - mask: reference adds +1.0 to logits of keys j < mask[b] before softmax.
    softmax(l + m) = e^m * e^l / sum  ->  fold e^m into V rows (and into the
    softmax-sum ones column), so masking costs nothing per tile.
  - softmax sums come from an extra ones column appended to V (head_dim 73);
    no vector reductions at all.
  - per-head K^T tiles are zero-padded to full 128-partition chunks so every
    matmul operand sits at base partition 0 (tile_position constraint).
  - normalization (1/sum) is applied after transposing per-head output to
    natural orientation, where rows are partitions and tensor_scalar works.

Changes vs the original baseline (381us -> ~319us at full clock):
  - weights/activations stored partition-major in DRAM so each tensor loads
    with a few large DMAs (contiguous per-partition runs) instead of 9 each;
    wq/wk additionally output-chunk-major so the first q-proj / K-proj
    column chains are gated by one 295KB DMA, not the full weight.
  - PSUM tag budgeting: lps+qps "big" (4 bufs), ops/onp/tps "att" (2),
    yps/kps/vps "yk" (2) = 8 banks.
  - two-stage software pipelining in the head loop: logits+exp for head h,
    PV+psum-evacuation for h-1, transpose+normalize for h-2. Removes PE
    head-of-line blocking on the ACT exp chain and on the DVE oT cast.
  - the 4 per-head 1/sum multiplies merged into one tensor_tensor with a
    stride-0 broadcast AP (onat is one [128, 4, D] tile).
  - oTc/ysb copies on DVE (with one ysb chunk on ACT) — out-proj phase was
    ACT-bound; kv K/V chunk emission interleaved.
  - out-proj loops c-outer so each oTc stationary is reused for multiple
    f-chunks; last group's yps tiles borrow the idle "big" slots.
  - output stored bf16 (halves output DMA; host casts back to f32).
  - fp8 was evaluated and REJECTED: host simulation gives rel err 4.5e-2+
    for any fp8 matmul path vs the 2e-2 budget (bf16 pipeline is 7.4e-3).
"""

import sys

sys.path.insert(0, "/opt/trn_rl_repo")

import ml_dtypes
import numpy as np

import concourse.bass as bass  # noqa: F401  (engine types via nc)
import concourse.mybir as mybir
import concourse.tile as tile
from concourse import bacc
from concourse.bass_utils import run_bass_kernel_spmd
from concourse.masks import make_identity

BF16 = mybir.dt.bfloat16
F32 = mybir.dt.float32
NPBF16 = ml_dtypes.bfloat16
AF = mybir.ActivationFunctionType

B, NSEQ, MKEY, D, H, DH = 4, 4096, 300, 1152, 16, 72
NCORES = 8
C = D // 128  # 9 feature chunks
KC = 3  # key chunks, keys padded 300 -> 384
MP = KC * 128
RG = 512  # query rows per group
SCALE = 1.0 / float(np.sqrt(DH))
ROWS_PER_CORE = B * NSEQ // NCORES  # 2048

LAST_EXEC_NS = None
LAST_RESULT = None


def _head_segs(h):
    """Feature range [72h, 72h+72) of head h split at 128-chunk boundaries.

    Returns [(chunk, lo, hi)] with chunk-local partition range [lo, hi)."""
    f0, f1 = DH * h, DH * h + DH
    segs = []
    c = f0 // 128
    while c * 128 < f1:
        lo = max(f0, c * 128) - c * 128
        hi = min(f1, (c + 1) * 128) - c * 128
        segs.append((c, lo, hi))
        c += 1
    return segs


def _chunk_segs(c):
    """[(h, i, lo, hi)] head segments living in feature chunk c."""
    out = []
    for h in range(H):
        for i, (hc, lo, hi) in enumerate(_head_segs(h)):
            if hc == c:
                out.append((h, i, lo, hi))
    return out


# flat order of all (head, segment) pairs; column index into the hmask input
_ALL_SEGS = [(h, i) for h in range(H) for i in range(len(_head_segs(h)))]
_SEG_IDX = {hs: s for s, hs in enumerate(_ALL_SEGS)}
NSEG = len(_ALL_SEGS)


def _hmask_host():
    """[128, NSEG] f32: column (h,i) is 1.0 on the chunk-local partitions of
    that head segment, 0 elsewhere. Engine ops can't address SBUF at
    non-32-aligned partition bases, so head extraction is done as a
    full-chunk copy multiplied by this per-partition mask."""
    m = np.zeros((128, NSEG), np.float32)
    for h in range(H):
        for i, (_, lo, hi) in enumerate(_head_segs(h)):
            m[lo:hi, _SEG_IDX[(h, i)]] = 1.0
    return m


def build_program(rpc=ROWS_PER_CORE, has_bq=False, has_bk=False, has_bv=False, has_bp=False):
    nc = bacc.Bacc()

    groups = rpc // RG
    tiles_per_group = RG // 128
    kn = [128, 128, MKEY - 256]  # real keys per key chunk

    # partition-major layouts: one big DMA per tensor
    xT_d = nc.dram_tensor("xT", [128, groups, C, RG], BF16, kind="ExternalInput")
    condT_d = nc.dram_tensor("condT", [128, C, MKEY], BF16, kind="ExternalInput")
    # wq is output-chunk-major: wq[c][p][k][q] = Wq[k*128+p, c*128+q], so the
    # first q-proj column chain only waits for one 295KB DMA (plus xT)
    wq_d = nc.dram_tensor("wq", [C, 128, C, 128], BF16, kind="ExternalInput")
    wk_d = nc.dram_tensor("wk", [C, 128, C, 128], BF16, kind="ExternalInput")
    wv_d = nc.dram_tensor("wv", [128, C, D], BF16, kind="ExternalInput")
    wp_d = nc.dram_tensor("wp", [128, C, D], BF16, kind="ExternalInput")
    bq_d = nc.dram_tensor("bq", [128, C], F32, kind="ExternalInput")
    bk_d = nc.dram_tensor("bk", [128, C], F32, kind="ExternalInput")
    bv_d = nc.dram_tensor("bv", [1, D], BF16, kind="ExternalInput")
    bp_d = nc.dram_tensor("bp", [1, D], BF16, kind="ExternalInput")
    vs_d = nc.dram_tensor("vscale", [128, KC], F32, kind="ExternalInput")
    hm_d = nc.dram_tensor("hmask", [128, NSEG], F32, kind="ExternalInput")
    out_d = nc.dram_tensor("out", [rpc, D], BF16, kind="ExternalOutput")

    YCH = [(0, 384), (384, 768), (768, 1152)]

    with tile.TileContext(nc) as tc:
        with (
            tc.tile_pool(name="const", bufs=1) as cpool,
            tc.tile_pool(name="ps", bufs=3, space="PSUM") as psp,
        ):
            # persistent weights / constants (split big loads in thirds so the
            # first q-proj matmuls can start after ~1/3 of the bytes land)
            wq_sb = cpool.tile([128, C, D], BF16)
            wp_sb = cpool.tile([128, C, D], BF16)
            ident = cpool.tile([128, 128], BF16)
            make_identity(nc, ident[:])
            # vs/hm are tiny and first needed in the kv phase; issue their
            # DMAs after the startup-critical wq/xT loads (see q_proj)
            vs_sb = cpool.tile([128, KC], F32)
            hm_sb = cpool.tile([128, NSEG], F32)
            if has_bq:
                bq_sb = cpool.tile([128, C], F32)
                nc.sync.dma_start(bq_sb[:], bq_d[:])
            if has_bk:
                bk_sb = cpool.tile([128, C], F32)
                nc.sync.dma_start(bk_sb[:], bk_d[:])
            if has_bp:
                bp_sb = cpool.tile([1, D], BF16)
                nc.sync.dma_start(bp_sb[:], bp_d[:])
            if has_bv or has_bp:
                ones_sb = cpool.tile([1, 128], BF16)
                nc.gpsimd.memset(ones_sb[:], 1.0)

            # V in natural orientation [key, head, dim+1]; fake keys stay 0,
            # col 72 holds e^mask (ones column pre-scaled by the mask factor)
            v_sb = cpool.tile([128, KC, H, DH + 1], BF16)
            nc.gpsimd.memset(v_sb[:], 0.0)
            kTz = {}
            for h in range(H):
                for i in range(len(_head_segs(h))):
                    t = cpool.tile([128, MP], BF16, name=f"kTz_{h}_{i}")
                    nc.gpsimd.memset(t[:], 0.0)
                    kTz[(h, i)] = t

            kT72 = {}
            for h in range(H):
                if len(_head_segs(h)) == 2:
                    t = cpool.tile([DH + 4, MP], BF16, name=f"kT72_{h}")
                    nc.gpsimd.memset(t[:], 0.0)
                    kT72[h] = t

            # ---- streaming: q-proj, attention, out-proj ----
            with tc.tile_pool(name="xq", bufs=2) as xqpool:
                qts = {}
                first = [True]

                def q_proj(g):
                    xT_sb = xqpool.tile([128, C, RG], BF16, name="xT", tag="xT")
                    if first[0]:
                        # startup: first q-proj output chunk is gated only by
                        # wq col-chunk 0 and xT; remaining wq chunks stream in
                        nc.sync.dma_start(wq_sb[:, :, 0:128], wq_d[0])
                        for k0, k1 in [(0, 3), (3, 6), (6, C)]:
                            nc.sync.dma_start(xT_sb[:, k0:k1, :], xT_d[:, g, k0:k1, :])
                        for c in range(1, C):
                            nc.sync.dma_start(
                                wq_sb[:, :, c * 128 : (c + 1) * 128], wq_d[c]
                            )
                        nc.sync.dma_start(vs_sb[:], vs_d[:])
                        nc.sync.dma_start(hm_sb[:], hm_d[:])
                        first[0] = False
                    else:
                        nc.sync.dma_start(xT_sb[:], xT_d[:, g])
                    qT_sb = xqpool.tile([128, C, RG], BF16, name="qT", tag="qT")
                    for c in range(C):
                        qps = psp.tile([128, RG], F32, name="qps", tag="big", bufs=4)
                        for k in range(C):
                            nc.tensor.matmul(
                                qps[:],
                                wq_sb[:, k, c * 128 : (c + 1) * 128],
                                xT_sb[:, k, :],
                                start=(k == 0),
                                stop=(k == C - 1),
                            )
                        if has_bq:
                            nc.scalar.activation(
                                qT_sb[:, c, :], qps[:], AF.Identity, bias=bq_sb[:, c : c + 1]
                            )
                        else:
                            nc.vector.tensor_copy(qT_sb[:, c, :], qps[:])
                    qts[g] = qT_sb

                q_proj(0)
                # ---- kv projection (weights in a scoped SBUF pool) ----
                with tc.tile_pool(name="kvw", bufs=1) as kvpool:
                    condT_sb = kvpool.tile([128, C, MKEY], BF16)
                    wk_sb = kvpool.tile([128, C, D], BF16)
                    wv_sb = kvpool.tile([128, C, D], BF16)
                    nc.sync.dma_start(condT_sb[:], condT_d[:])
                    # interleave wk (by output chunk) and wv (by vch column
                    # group) so the kv K/V chains start as bytes land
                    vch_bounds = [(0, 360), (360, 720), (720, 1080), (1080, 1152)]
                    for c in range(C):
                        nc.sync.dma_start(wk_sb[:, :, c * 128 : (c + 1) * 128], wk_d[c])
                        if c < len(vch_bounds):
                            f0, f1 = vch_bounds[c]
                            nc.sync.dma_start(wv_sb[:, :, f0:f1], wv_d[:, :, f0:f1])
                    nc.sync.dma_start(wp_sb[:], wp_d[:])
                    if has_bv:
                        bv_sb = kvpool.tile([1, D], BF16)
                        nc.sync.dma_start(bv_sb[:], bv_d[:])

                    # K^T in feature-chunk orientation -> zero-padded head tiles
                    def emit_k(c):
                        kps = psp.tile([128, MKEY], F32, name="kps", tag="yk", bufs=2)
                        for k in range(C):
                            nc.tensor.matmul(
                                kps[:],
                                wk_sb[:, k, c * 128 : (c + 1) * 128],
                                condT_sb[:, k, :],
                                start=(k == 0),
                                stop=(k == C - 1),
                            )
                        for h, i, _lo, _hi in _chunk_segs(c):
                            s = _SEG_IDX[(h, i)]
                            if has_bk:
                                nc.vector.tensor_scalar(
                                    kTz[(h, i)][:, 0:MKEY],
                                    kps[:],
                                    bk_sb[:, c : c + 1],
                                    hm_sb[:, s : s + 1],
                                    op0=mybir.AluOpType.add,
                                    op1=mybir.AluOpType.mult,
                                )
                            else:
                                nc.vector.tensor_scalar_mul(
                                    kTz[(h, i)][:, 0:MKEY], kps[:], hm_sb[:, s : s + 1]
                                )

                    # V natural [keys, feat], head-aligned 360-wide chunks
                    vch = [(0, 360), (360, 720), (720, 1080), (1080, 1152)]

                    def emit_v(kc, f0, f1):
                        vps = psp.tile([128, f1 - f0], F32, name="vps", tag="yk", bufs=2)
                        for k in range(C):
                            nc.tensor.matmul(
                                vps[0 : kn[kc], :],
                                condT_sb[:, k, kc * 128 : kc * 128 + kn[kc]],
                                wv_sb[:, k, f0:f1],
                                start=(k == 0),
                                stop=(k == C - 1 and not has_bv),
                            )
                        if has_bv:
                            nc.tensor.matmul(
                                vps[0 : kn[kc], :],
                                ones_sb[0:1, 0 : kn[kc]],
                                bv_sb[0:1, f0:f1],
                                start=False,
                                stop=True,
                            )
                        for h in range(f0 // DH, f1 // DH):
                            d0 = h * DH - f0
                            nc.vector.tensor_scalar_mul(
                                v_sb[0 : kn[kc], kc, h, 0:DH],
                                vps[0 : kn[kc], d0 : d0 + DH],
                                vs_sb[0 : kn[kc], kc : kc + 1],
                            )

                    # interleave K and V chunks: the PE streams one chain
                    # while the DVE drains the other's extraction muls
                    vlist = [(kc, f0, f1) for kc in range(KC) for (f0, f1) in vch]
                    for j in range(max(C, len(vlist))):
                        if j < C:
                            emit_k(j)
                        if j < len(vlist):
                            emit_v(*vlist[j])
                    for kc in range(KC):
                        for h in range(H):
                            nc.any.tensor_copy(
                                v_sb[0 : kn[kc], kc, h, DH : DH + 1],
                                vs_sb[0 : kn[kc], kc : kc + 1],
                            )

                    for h, t in kT72.items():
                        (c0, lo0, hi0), (c1, lo1, hi1) = _head_segs(h)
                        n0 = hi0 - lo0
                        nc.sync.dma_start(t[0:n0, 0:MKEY], kTz[(h, 0)][lo0:hi0, 0:MKEY])
                        nc.sync.dma_start(t[n0 : n0 + (hi1 - lo1), 0:MKEY], kTz[(h, 1)][lo1:hi1, 0:MKEY])
                with (
                    tc.tile_pool(name="att", bufs=4) as apool,
                    tc.tile_pool(name="outp", bufs=2) as opool,
                ):
                    def attention(g):
                        # the last group's out-proj has no successor work to
                        # hide psum-evacuation waits; borrow the (idle) lps/qps
                        # slots for deeper yps pipelining there
                        ytag, ybufs = ("big", 4) if g == groups - 1 else ("yk", 2)
                        qT_sb = qts.pop(g)
                        qTg = {}
                        for h in sorted(kT72):
                            (c0, lo0, hi0), (c1, lo1, hi1) = _head_segs(h)
                            n0 = hi0 - lo0
                            t = opool.tile([DH + 4, RG], BF16, name=f"qTg{h}", tag=f"qTg{h}")
                            nc.sync.dma_start(t[0:n0, :], qT_sb[lo0:hi0, c0, :])
                            nc.sync.dma_start(t[n0:DH, :], qT_sb[lo1:hi1, c1, :])
                            qTg[h] = t

                        # one [128, rt, feat] natural-orientation tile per group
                        onat = opool.tile(
                            [128, tiles_per_group, D], BF16, name="onat", tag="onat"
                        )
                        head_order = [h for h in range(H) if h not in kT72] + sorted(kT72)

                        def head_pv(h, expT):
                            """PV + psum evacuation for head h (one head behind
                            the logits/exp front)."""
                            ops = psp.tile([DH + 1, RG], F32, name="ops", tag="att", bufs=2)
                            for kc in range(KC):
                                nc.tensor.matmul(
                                    ops[:],
                                    v_sb[:, kc, h, :],
                                    expT[:, kc, :],
                                    start=(kc == 0),
                                    stop=(kc == KC - 1),
                                )
                            oT_sb = apool.tile([DH + 1, RG], BF16, name="oT", tag="oT")
                            nc.vector.tensor_copy(oT_sb[:], ops[:])
                            return oT_sb

                        def head_norm(h, oT_sb):
                            """transpose + normalize for head h (two heads
                            behind, so the oT cast has a full head-period)."""
                            onp = psp.tile(
                                [128, tiles_per_group, DH + 4], BF16, name="onp", tag="att", bufs=2
                            )
                            for rt in range(tiles_per_group):
                                nc.tensor.transpose(
                                    onp[:, rt, 0 : DH + 1],
                                    oT_sb[:, rt * 128 : (rt + 1) * 128],
                                    ident[0 : DH + 1, 0 : DH + 1],
                                )
                            inv = apool.tile([128, tiles_per_group], F32, name="inv", tag="inv")
                            nc.vector.reciprocal(inv[:], onp[:, :, DH])
                            # all 4 row-tiles normalized in one op (stride-0
                            # broadcast of inv along the feature dim)
                            nc.vector.tensor_mul(
                                onat[:, :, h * DH : (h + 1) * DH],
                                onp[:, :, 0:DH],
                                inv[:].unsqueeze(2).broadcast_to([128, tiles_per_group, DH]),
                            )

                        p1 = p2 = None
                        for h in head_order:
                            segs = _head_segs(h)
                            expT = apool.tile([128, KC, RG], BF16, name="expT", tag="expT")
                            for kc in range(KC):
                                lps = psp.tile([128, RG], F32, name="lps", tag="big", bufs=4)
                                if h in kT72:
                                    nc.tensor.matmul(
                                        lps[:],
                                        kT72[h][0:DH, kc * 128 : (kc + 1) * 128],
                                        qTg[h][0:DH, :],
                                        start=True,
                                        stop=True,
                                    )
                                else:
                                    (c, lo, hi) = segs[0]
                                    nc.tensor.matmul(
                                        lps[:],
                                        kTz[(h, 0)][:, kc * 128 : (kc + 1) * 128],
                                        qT_sb[:, c, :],
                                        start=True,
                                        stop=True,
                                    )
                                nc.scalar.activation(expT[:, kc, :], lps[:], AF.Exp, scale=SCALE)
                            if p1 is not None:
                                oT = head_pv(*p1)
                                if p2 is not None:
                                    head_norm(*p2)
                                p2 = (p1[0], oT)
                            p1 = (h, expT)
                        oT = head_pv(*p1)
                        head_norm(*p2)
                        head_norm(p1[0], oT)

                        for rt in range(tiles_per_group):
                            grt = g * tiles_per_group + rt
                            oTc_sb = opool.tile([128, C, 128], BF16, name="oTc", tag="oTc")
                            for c3 in range(C // 3):
                                tps = psp.tile([128, 3, 128], BF16, name="tps", tag="att", bufs=2)
                                for j in range(3):
                                    c = c3 * 3 + j
                                    nc.tensor.transpose(
                                        tps[:, j, :],
                                        onat[:, rt, c * 128 : (c + 1) * 128],
                                        ident[:],
                                    )
                                nc.vector.tensor_copy(oTc_sb[:, c3 * 3 : c3 * 3 + 3, :], tps[:])

                            ysb = opool.tile([128, D], BF16, name="ysb", tag="y")
                            ypss = [
                                psp.tile([128, f1 - f0], F32, name=f"yps{fi}", tag=ytag, bufs=ybufs)
                                for fi, (f0, f1) in enumerate(YCH[:2])
                            ]
                            # c-outer: each oTc stationary reused for 2 f-chunks
                            for c in range(C):
                                for fi, (f0, f1) in enumerate(YCH[:2]):
                                    nc.tensor.matmul(
                                        ypss[fi][:],
                                        oTc_sb[:, c, :],
                                        wp_sb[:, c, f0:f1],
                                        start=(c == 0),
                                        stop=(c == C - 1 and not has_bp),
                                    )
                            for fi, (f0, f1) in enumerate(YCH[:2]):
                                if has_bp:
                                    nc.tensor.matmul(
                                        ypss[fi][:],
                                        ones_sb[0:1, :],
                                        bp_sb[0:1, f0:f1],
                                        start=False,
                                        stop=True,
                                    )
                                # alternate psum-evacuation engines so yps
                                # recycling isn't gated on one engine
                                if fi == 1:
                                    nc.scalar.copy(ysb[:, f0:f1], ypss[fi][:])
                                else:
                                    nc.vector.tensor_copy(ysb[:, f0:f1], ypss[fi][:])
                                nc.sync.dma_start(
                                    out_d[grt * 128 : (grt + 1) * 128, f0:f1], ysb[:, f0:f1]
                                )
                            f0, f1 = YCH[2]
                            yps2 = psp.tile([128, f1 - f0], F32, name="yps2", tag=ytag, bufs=ybufs)
                            for c in range(C):
                                nc.tensor.matmul(
                                    yps2[:],
                                    oTc_sb[:, c, :],
                                    wp_sb[:, c, f0:f1],
                                    start=(c == 0),
                                    stop=(c == C - 1 and not has_bp),
                                )
                            if has_bp:
                                nc.tensor.matmul(
                                    yps2[:],
                                    ones_sb[0:1, :],
                                    bp_sb[0:1, f0:f1],
                                    start=False,
                                    stop=True,
                                )
                            nc.vector.tensor_copy(ysb[:, f0:f1], yps2[:])
                            nc.sync.dma_start(
                                out_d[grt * 128 : (grt + 1) * 128, f0:f1], ysb[:, f0:f1]
                            )
                    for g in range(groups):
                        if g + 1 < groups:
                            q_proj(g + 1)
                        attention(g)

    nc.compile()
    return nc


_programs = {}


def _get_program(key):
    if key not in _programs:
        _programs[key] = build_program(*key)
    return _programs[key]


def make_in_maps(x, cond, mask, Wq, bq, Wkv, bkv, Wp, bp, rpc=ROWS_PER_CORE, ncores=NCORES):
    """Host-side shard + relayout. Returns (in_maps, flags)."""
    x = np.asarray(x, np.float32)
    cond = np.asarray(cond, np.float32)
    mask = np.asarray(mask)
    Wq = np.asarray(Wq, np.float32)
    Wkv = np.asarray(Wkv, np.float32)
    Wp = np.asarray(Wp, np.float32)
    bq = np.asarray(bq, np.float32)
    bkv = np.asarray(bkv, np.float32)
    bp = np.asarray(bp, np.float32)

    def pmajor(w):  # [D, D2] -> [128, C, D2] partition-major
        d2 = w.shape[1]
        return np.ascontiguousarray(
            w.astype(NPBF16).reshape(C, 128, d2).transpose(1, 0, 2)
        )

    # [C_out, 128, C_in, 128]: wq[c, p, k, q] = Wq[k*128+p, c*128+q]
    wq = np.ascontiguousarray(
        Wq.astype(NPBF16).reshape(C, 128, C, 128).transpose(2, 1, 0, 3)
    )
    wk = np.ascontiguousarray(
        Wkv[:, :D].astype(NPBF16).reshape(C, 128, C, 128).transpose(2, 1, 0, 3)
    )
    wv = pmajor(Wkv[:, D:])
    wp = pmajor(Wp)
    bq_a = np.ascontiguousarray(bq.reshape(C, 128).T)
    bk_a = np.ascontiguousarray(bkv[:D].reshape(C, 128).T)
    bv_a = bkv[D:].astype(NPBF16).reshape(1, D)
    bp_a = bp.astype(NPBF16).reshape(1, D)

    flags = (rpc, bool(bq.any()), bool(bkv[:D].any()), bool(bkv[D:].any()), bool(bp.any()))
    hmask = _hmask_host()

    halves = NSEQ // rpc
    groups = rpc // RG
    in_maps = []
    for core in range(ncores):
        b, half = core // halves, core % halves
        rows = slice(half * rpc, (half + 1) * rpc)
        # [128, G, C, RG]: xT[p, g, c, r] = x[b, g*RG+r, c*128+p]
        xT = np.ascontiguousarray(
            x[b, rows].T.astype(NPBF16).reshape(C, 128, groups, RG).transpose(1, 2, 0, 3)
        )
        condT = np.ascontiguousarray(
            cond[b].T.astype(NPBF16).reshape(C, 128, MKEY).transpose(1, 0, 2)
        )
        mv = (np.arange(MP) < int(mask[b])).astype(np.float32)
        vscale = np.ascontiguousarray(np.exp(mv).reshape(KC, 128).T)
        in_maps.append(
            {
                "xT": xT,
                "condT": condT,
                "wq": wq,
                "wk": wk,
                "wv": wv,
                "wp": wp,
                "bq": bq_a,
                "bk": bk_a,
                "bv": bv_a,
                "bp": bp_a,
                "vscale": vscale,
                "hmask": hmask,
            }
        )
    return in_maps, flags


def kernel(x, cond, mask, Wq, bq, Wkv, bkv, Wp, bp):
    global LAST_EXEC_NS
    import os
    import time

    in_maps, flags = make_in_maps(x, cond, mask, Wq, bq, Wkv, bkv, Wp, bp)
    nc = _get_program(flags)
    trace = bool(os.environ.get("BASS_KERNEL_TRACE"))
    res = None
    for attempt in range(3):
        try:
            res = run_bass_kernel_spmd(nc, in_maps, list(range(NCORES)), trace=trace)
            break
        except Exception:
            if attempt == 2:
                raise
            time.sleep(10)
    LAST_EXEC_NS = res.exec_time_ns
    globals()["LAST_RESULT"] = res

    rpc = flags[0]
    halves = NSEQ // rpc
    out = np.empty((B, NSEQ, D), np.float32)
    for core in range(NCORES):
        b, half = core // halves, core % halves
        out[b, half * rpc : (half + 1) * rpc] = res.results[core]["out"].astype(np.float32)
    return out



# revision 6
# speedup vs baseline: 1.0844x; 1.0844x over previous
"""Multi-head cross-attention Trainium2 kernel (8-core SPMD, data-parallel).

Shards (batch=4) x (seq halves) across 8 NeuronCores; each core runs the
full q/kv/attention/out-proj pipeline for its 2048 query rows in bf16 with
fp32 PSUM accumulation.

Key tricks:
  - mask: reference adds +1.0 to logits of keys j < mask[b] before softmax.
    softmax(l + m) = e^m * e^l / sum  ->  fold e^m into V rows (and into the
    softmax-sum ones column), so masking costs nothing per tile.
  - softmax sums come from an extra ones column appended to V (head_dim 73);
    no vector reductions at all.
  - per-head K^T tiles are zero-padded to full 128-partition chunks so every
    matmul operand sits at base partition 0 (tile_position constraint).
  - normalization (1/sum) is applied after transposing per-head output to
    natural orientation, where rows are partitions and tensor_scalar works.

Round 2 (327us -> target ~280us):
  - PV operand swap: exp is the STATIONARY operand ([keys, row-tile]) and V
    the moving one ([keys, 73]); output lands row-major so normalization is
    a per-partition multiply. Removes the per-head output transposes and the
    oT psum-evacuation cast, and cuts PV moving rows from 1536 to 876 per
    head-group (stationary reloads are hidden: measured 39ns issue spacing
    on 73-row-moving matmuls).
  - q-proj for group g+1 is interleaved one column-chain at a time into
    attention(g)'s head loop: the exp chain on ACT (~1.7us/head) exceeds the
    PE's per-head work (~1.1us), so the PE fills the gap with next-group
    q-proj matmuls instead of idling.

Changes vs the original baseline (381us -> ~319us at full clock):
  - weights/activations stored partition-major in DRAM so each tensor loads
    with a few large DMAs (contiguous per-partition runs) instead of 9 each;
    wq/wk additionally output-chunk-major so the first q-proj / K-proj
    column chains are gated by one 295KB DMA, not the full weight.
  - PSUM tag budgeting: lps+qps "big" (4 bufs), ops/onp/tps "att" (2),
    yps/kps/vps "yk" (2) = 8 banks.
  - two-stage software pipelining in the head loop: logits+exp for head h,
    PV+psum-evacuation for h-1, transpose+normalize for h-2. Removes PE
    head-of-line blocking on the ACT exp chain and on the DVE oT cast.
  - the 4 per-head 1/sum multiplies merged into one tensor_tensor with a
    stride-0 broadcast AP (onat is one [128, 4, D] tile).
  - oTc/ysb copies on DVE (with one ysb chunk on ACT) — out-proj phase was
    ACT-bound; kv K/V chunk emission interleaved.
  - out-proj loops c-outer so each oTc stationary is reused for multiple
    f-chunks; last group's yps tiles borrow the idle "big" slots.
  - output stored bf16 (halves output DMA; host casts back to f32).
  - fp8 was evaluated and REJECTED: host simulation gives rel err 4.5e-2+
    for any fp8 matmul path vs the 2e-2 budget (bf16 pipeline is 7.4e-3).
"""

import sys

sys.path.insert(0, "/opt/trn_rl_repo")

import ml_dtypes
import numpy as np

import concourse.bass as bass  # noqa: F401  (engine types via nc)
import concourse.mybir as mybir
import concourse.tile as tile
from concourse import bacc
from concourse.bass_utils import run_bass_kernel_spmd
from concourse.masks import make_identity

BF16 = mybir.dt.bfloat16
F32 = mybir.dt.float32
NPBF16 = ml_dtypes.bfloat16
AF = mybir.ActivationFunctionType

B, NSEQ, MKEY, D, H, DH = 4, 4096, 300, 1152, 16, 72
NCORES = 8
C = D // 128  # 9 feature chunks
KC = 3  # key chunks, keys padded 300 -> 384
MP = KC * 128
RG = 512  # query rows per group
SCALE = 1.0 / float(np.sqrt(DH))
ROWS_PER_CORE = B * NSEQ // NCORES  # 2048

LAST_EXEC_NS = None
LAST_RESULT = None


def _head_segs(h):
    """Feature range [72h, 72h+72) of head h split at 128-chunk boundaries.

    Returns [(chunk, lo, hi)] with chunk-local partition range [lo, hi)."""
    f0, f1 = DH * h, DH * h + DH
    segs = []
    c = f0 // 128
    while c * 128 < f1:
        lo = max(f0, c * 128) - c * 128
        hi = min(f1, (c + 1) * 128) - c * 128
        segs.append((c, lo, hi))
        c += 1
    return segs


def _chunk_segs(c):
    """[(h, i, lo, hi)] head segments living in feature chunk c."""
    out = []
    for h in range(H):
        for i, (hc, lo, hi) in enumerate(_head_segs(h)):
            if hc == c:
                out.append((h, i, lo, hi))
    return out


# flat order of all (head, segment) pairs; column index into the hmask input
_ALL_SEGS = [(h, i) for h in range(H) for i in range(len(_head_segs(h)))]
_SEG_IDX = {hs: s for s, hs in enumerate(_ALL_SEGS)}
NSEG = len(_ALL_SEGS)


def _hmask_host():
    """[128, NSEG] f32: column (h,i) is 1.0 on the chunk-local partitions of
    that head segment, 0 elsewhere. Engine ops can't address SBUF at
    non-32-aligned partition bases, so head extraction is done as a
    full-chunk copy multiplied by this per-partition mask."""
    m = np.zeros((128, NSEG), np.float32)
    for h in range(H):
        for i, (_, lo, hi) in enumerate(_head_segs(h)):
            m[lo:hi, _SEG_IDX[(h, i)]] = 1.0
    return m


def build_program(rpc=ROWS_PER_CORE, has_bq=False, has_bk=False, has_bv=False, has_bp=False):
    nc = bacc.Bacc()

    groups = rpc // RG
    tiles_per_group = RG // 128
    kn = [128, 128, MKEY - 256]  # real keys per key chunk

    # partition-major layouts: one big DMA per tensor
    xT_d = nc.dram_tensor("xT", [128, groups, C, RG], BF16, kind="ExternalInput")
    condT_d = nc.dram_tensor("condT", [128, C, MKEY], BF16, kind="ExternalInput")
    # wq is output-chunk-major: wq[c][p][k][q] = Wq[k*128+p, c*128+q], so the
    # first q-proj column chain only waits for one 295KB DMA (plus xT)
    wq_d = nc.dram_tensor("wq", [C, 128, C, 128], BF16, kind="ExternalInput")
    wk_d = nc.dram_tensor("wk", [C, 128, C, 128], BF16, kind="ExternalInput")
    wv_d = nc.dram_tensor("wv", [128, C, D], BF16, kind="ExternalInput")
    wp_d = nc.dram_tensor("wp", [128, C, D], BF16, kind="ExternalInput")
    bq_d = nc.dram_tensor("bq", [128, C], F32, kind="ExternalInput")
    bk_d = nc.dram_tensor("bk", [128, C], F32, kind="ExternalInput")
    bv_d = nc.dram_tensor("bv", [1, D], BF16, kind="ExternalInput")
    bp_d = nc.dram_tensor("bp", [1, D], BF16, kind="ExternalInput")
    vs_d = nc.dram_tensor("vscale", [128, KC], F32, kind="ExternalInput")
    hm_d = nc.dram_tensor("hmask", [128, NSEG], F32, kind="ExternalInput")
    out_d = nc.dram_tensor("out", [rpc, D], BF16, kind="ExternalOutput")

    YCH = [(0, 384), (384, 768), (768, 1152)]

    with tile.TileContext(nc) as tc:
        with (
            tc.tile_pool(name="const", bufs=1) as cpool,
            tc.tile_pool(name="ps", bufs=3, space="PSUM") as psp,
        ):
            # persistent weights / constants (split big loads in thirds so the
            # first q-proj matmuls can start after ~1/3 of the bytes land)
            wq_sb = cpool.tile([128, C, D], BF16)
            wp_sb = cpool.tile([128, C, D], BF16)
            ident = cpool.tile([128, 128], BF16)
            make_identity(nc, ident[:])
            # vs/hm are tiny and first needed in the kv phase; issue their
            # DMAs after the startup-critical wq/xT loads (see q_proj)
            vs_sb = cpool.tile([128, KC], F32)
            hm_sb = cpool.tile([128, NSEG], F32)
            if has_bq:
                bq_sb = cpool.tile([128, C], F32)
                nc.sync.dma_start(bq_sb[:], bq_d[:])
            if has_bk:
                bk_sb = cpool.tile([128, C], F32)
                nc.sync.dma_start(bk_sb[:], bk_d[:])
            if has_bp:
                bp_sb = cpool.tile([1, D], BF16)
                nc.sync.dma_start(bp_sb[:], bp_d[:])
            if has_bv or has_bp:
                ones_sb = cpool.tile([1, 128], BF16)
                nc.gpsimd.memset(ones_sb[:], 1.0)

            # V in natural orientation [key, head, dim+1]; fake keys stay 0,
            # col 72 holds e^mask (ones column pre-scaled by the mask factor)
            v_sb = cpool.tile([128, KC, H, DH + 1], BF16)
            nc.gpsimd.memset(v_sb[:], 0.0)
            kTz = {}
            for h in range(H):
                for i in range(len(_head_segs(h))):
                    t = cpool.tile([128, MP], BF16, name=f"kTz_{h}_{i}")
                    nc.gpsimd.memset(t[:], 0.0)
                    kTz[(h, i)] = t

            kT72 = {}
            for h in range(H):
                if len(_head_segs(h)) == 2:
                    t = cpool.tile([DH + 4, MP], BF16, name=f"kT72_{h}")
                    nc.gpsimd.memset(t[:], 0.0)
                    kT72[h] = t

            # ---- streaming: q-proj, attention, out-proj ----
            with tc.tile_pool(name="xq", bufs=2) as xqpool:
                qts = {}
                first = [True]

                def q_proj_chunks(g):
                    """Start the xT DMA for group g and return one thunk per
                    q-proj output chunk (9 matmuls + psum evacuation each)."""
                    xT_sb = xqpool.tile([128, C, RG], BF16, name="xT", tag="xT")
                    if first[0]:
                        # startup: first q-proj output chunk is gated only by
                        # wq col-chunk 0 and xT; remaining wq chunks stream in
                        nc.sync.dma_start(wq_sb[:, :, 0:128], wq_d[0])
                        for k0, k1 in [(0, 3), (3, 6), (6, C)]:
                            nc.sync.dma_start(xT_sb[:, k0:k1, :], xT_d[:, g, k0:k1, :])
                        for c in range(1, C):
                            nc.sync.dma_start(
                                wq_sb[:, :, c * 128 : (c + 1) * 128], wq_d[c]
                            )
                        nc.sync.dma_start(vs_sb[:], vs_d[:])
                        nc.sync.dma_start(hm_sb[:], hm_d[:])
                        first[0] = False
                    else:
                        nc.sync.dma_start(xT_sb[:], xT_d[:, g])
                    qT_sb = xqpool.tile([128, C, RG], BF16, name="qT", tag="qT")
                    qts[g] = qT_sb

                    def chunk(c):
                        qps = psp.tile([128, RG], F32, name="qps", tag="big", bufs=4)
                        for k in range(C):
                            nc.tensor.matmul(
                                qps[:],
                                wq_sb[:, k, c * 128 : (c + 1) * 128],
                                xT_sb[:, k, :],
                                start=(k == 0),
                                stop=(k == C - 1),
                            )
                        if has_bq:
                            nc.scalar.activation(
                                qT_sb[:, c, :], qps[:], AF.Identity, bias=bq_sb[:, c : c + 1]
                            )
                        else:
                            nc.vector.tensor_copy(qT_sb[:, c, :], qps[:])

                    return [lambda c=c: chunk(c) for c in range(C)]

                def q_proj(g):
                    for f in q_proj_chunks(g):
                        f()

                q_proj(0)
                # ---- kv projection (weights in a scoped SBUF pool) ----
                with tc.tile_pool(name="kvw", bufs=1) as kvpool:
                    condT_sb = kvpool.tile([128, C, MKEY], BF16)
                    wk_sb = kvpool.tile([128, C, D], BF16)
                    wv_sb = kvpool.tile([128, C, D], BF16)
                    nc.sync.dma_start(condT_sb[:], condT_d[:])
                    # interleave wk (by output chunk) and wv (by vch column
                    # group) so the kv K/V chains start as bytes land
                    vch_bounds = [(0, 360), (360, 720), (720, 1080), (1080, 1152)]
                    for c in range(C):
                        nc.sync.dma_start(wk_sb[:, :, c * 128 : (c + 1) * 128], wk_d[c])
                        if c < len(vch_bounds):
                            f0, f1 = vch_bounds[c]
                            nc.sync.dma_start(wv_sb[:, :, f0:f1], wv_d[:, :, f0:f1])
                    nc.sync.dma_start(wp_sb[:], wp_d[:])
                    if has_bv:
                        bv_sb = kvpool.tile([1, D], BF16)
                        nc.sync.dma_start(bv_sb[:], bv_d[:])

                    # K^T in feature-chunk orientation -> zero-padded head tiles
                    def emit_k(c):
                        kps = psp.tile([128, MKEY], F32, name="kps", tag="yk", bufs=2)
                        for k in range(C):
                            nc.tensor.matmul(
                                kps[:],
                                wk_sb[:, k, c * 128 : (c + 1) * 128],
                                condT_sb[:, k, :],
                                start=(k == 0),
                                stop=(k == C - 1),
                            )
                        for h, i, _lo, _hi in _chunk_segs(c):
                            s = _SEG_IDX[(h, i)]
                            if has_bk:
                                nc.vector.tensor_scalar(
                                    kTz[(h, i)][:, 0:MKEY],
                                    kps[:],
                                    bk_sb[:, c : c + 1],
                                    hm_sb[:, s : s + 1],
                                    op0=mybir.AluOpType.add,
                                    op1=mybir.AluOpType.mult,
                                )
                            else:
                                nc.vector.tensor_scalar_mul(
                                    kTz[(h, i)][:, 0:MKEY], kps[:], hm_sb[:, s : s + 1]
                                )

                    # V natural [keys, feat], head-aligned 360-wide chunks
                    vch = [(0, 360), (360, 720), (720, 1080), (1080, 1152)]

                    def emit_v(kc, f0, f1):
                        vps = psp.tile([128, f1 - f0], F32, name="vps", tag="yk", bufs=2)
                        for k in range(C):
                            nc.tensor.matmul(
                                vps[0 : kn[kc], :],
                                condT_sb[:, k, kc * 128 : kc * 128 + kn[kc]],
                                wv_sb[:, k, f0:f1],
                                start=(k == 0),
                                stop=(k == C - 1 and not has_bv),
                            )
                        if has_bv:
                            nc.tensor.matmul(
                                vps[0 : kn[kc], :],
                                ones_sb[0:1, 0 : kn[kc]],
                                bv_sb[0:1, f0:f1],
                                start=False,
                                stop=True,
                            )
                        for h in range(f0 // DH, f1 // DH):
                            d0 = h * DH - f0
                            nc.vector.tensor_scalar_mul(
                                v_sb[0 : kn[kc], kc, h, 0:DH],
                                vps[0 : kn[kc], d0 : d0 + DH],
                                vs_sb[0 : kn[kc], kc : kc + 1],
                            )

                    # interleave K and V chunks: the PE streams one chain
                    # while the DVE drains the other's extraction muls
                    vlist = [(kc, f0, f1) for kc in range(KC) for (f0, f1) in vch]
                    for j in range(max(C, len(vlist))):
                        if j < C:
                            emit_k(j)
                        if j < len(vlist):
                            emit_v(*vlist[j])
                    for kc in range(KC):
                        for h in range(H):
                            nc.any.tensor_copy(
                                v_sb[0 : kn[kc], kc, h, DH : DH + 1],
                                vs_sb[0 : kn[kc], kc : kc + 1],
                            )

                    for h, t in kT72.items():
                        (c0, lo0, hi0), (c1, lo1, hi1) = _head_segs(h)
                        n0 = hi0 - lo0
                        nc.sync.dma_start(t[0:n0, 0:MKEY], kTz[(h, 0)][lo0:hi0, 0:MKEY])
                        nc.sync.dma_start(t[n0 : n0 + (hi1 - lo1), 0:MKEY], kTz[(h, 1)][lo1:hi1, 0:MKEY])
                with (
                    tc.tile_pool(name="att", bufs=4) as apool,
                    tc.tile_pool(name="outp", bufs=2) as opool,
                ):
                    def attention(g, fillers=()):
                        # the last group's out-proj has no successor work to
                        # hide psum-evacuation waits; borrow the (idle) lps/qps
                        # slots for deeper yps pipelining there
                        ytag, ybufs = ("big", 4) if g == groups - 1 else ("yk", 2)
                        fillers = list(fillers)
                        qT_sb = qts.pop(g)
                        qTg = {}
                        for h in sorted(kT72):
                            (c0, lo0, hi0), (c1, lo1, hi1) = _head_segs(h)
                            n0 = hi0 - lo0
                            t = opool.tile([DH + 4, RG], BF16, name=f"qTg{h}", tag=f"qTg{h}")
                            nc.sync.dma_start(t[0:n0, :], qT_sb[lo0:hi0, c0, :])
                            nc.sync.dma_start(t[n0:DH, :], qT_sb[lo1:hi1, c1, :])
                            qTg[h] = t

                        # one [128, rt, feat] natural-orientation tile per group
                        onat = opool.tile(
                            [128, tiles_per_group, D], BF16, name="onat", tag="onat"
                        )
                        head_order = [h for h in range(H) if h not in kT72] + sorted(kT72)

                        def head_pv(h, expT):
                            """PV for head h with exp as the STATIONARY operand
                            (one head behind the logits/exp front).

                            out[row, dim] = sum_k exp[k, row] * V[k, dim] comes
                            out row-major, so softmax normalization is a plain
                            per-partition multiply and no transposes are needed.
                            12 matmuls of 73 moving rows (876 cycles) replace
                            3 of 512 (1536); the measured issue rate of short-
                            moving matmuls shows stationary reloads are hidden.
                            """
                            ops = psp.tile(
                                [128, tiles_per_group, DH + 4], F32, name="ops", tag="att", bufs=2
                            )
                            # rt-OUTER so each accumulation group (kc chain) is
                            # contiguous: start=True clears has_written bits for
                            # the WHOLE bank, so interleaving groups within one
                            # bank corrupts earlier partial sums. A finished
                            # group's values survive later bit-clears (only the
                            # bits reset, not the data).
                            for rt in range(tiles_per_group):
                                for kc in range(KC):
                                    nc.tensor.matmul(
                                        ops[:, rt, 0 : DH + 1],
                                        expT[:, kc, rt * 128 : (rt + 1) * 128],
                                        v_sb[:, kc, h, :],
                                        start=(kc == 0),
                                        stop=(kc == KC - 1),
                                    )
                            inv = apool.tile([128, tiles_per_group], F32, name="inv", tag="inv")
                            nc.vector.reciprocal(inv[:], ops[:, :, DH])
                            # all 4 row-tiles normalized in one op (stride-0
                            # broadcast of inv along the feature dim)
                            nc.vector.tensor_mul(
                                onat[:, :, h * DH : (h + 1) * DH],
                                ops[:, :, 0:DH],
                                inv[:].unsqueeze(2).broadcast_to([128, tiles_per_group, DH]),
                            )

                        p1 = None
                        for hi, h in enumerate(head_order):
                            segs = _head_segs(h)
                            expT = apool.tile([128, KC, RG], BF16, name="expT", tag="expT")
                            for kc in range(KC):
                                lps = psp.tile([128, RG], F32, name="lps", tag="big", bufs=4)
                                if h in kT72:
                                    nc.tensor.matmul(
                                        lps[:],
                                        kT72[h][0:DH, kc * 128 : (kc + 1) * 128],
                                        qTg[h][0:DH, :],
                                        start=True,
                                        stop=True,
                                    )
                                else:
                                    (c, lo, hi) = segs[0]
                                    nc.tensor.matmul(
                                        lps[:],
                                        kTz[(h, 0)][:, kc * 128 : (kc + 1) * 128],
                                        qT_sb[:, c, :],
                                        start=True,
                                        stop=True,
                                    )
                                nc.scalar.activation(expT[:, kc, :], lps[:], AF.Exp, scale=SCALE)
                            if p1 is not None:
                                head_pv(*p1)
                            # ACT (3 exps, ~1.7us) outruns the PE's own work in
                            # a head slot (~1.1us); interleave one next-group
                            # q-proj column chain every other head so the PE
                            # stays busy instead of idling on the exp chain
                            if hi % 2 == 1 and fillers:
                                fillers.pop(0)()
                            p1 = (h, expT)
                        head_pv(*p1)
                        while fillers:
                            fillers.pop(0)()

                        for rt in range(tiles_per_group):
                            grt = g * tiles_per_group + rt
                            oTc_sb = opool.tile([128, C, 128], BF16, name="oTc", tag="oTc")
                            for c3 in range(C // 3):
                                tps = psp.tile([128, 3, 128], BF16, name="tps", tag="att", bufs=2)
                                for j in range(3):
                                    c = c3 * 3 + j
                                    nc.tensor.transpose(
                                        tps[:, j, :],
                                        onat[:, rt, c * 128 : (c + 1) * 128],
                                        ident[:],
                                    )
                                nc.vector.tensor_copy(oTc_sb[:, c3 * 3 : c3 * 3 + 3, :], tps[:])

                            ysb = opool.tile([128, D], BF16, name="ysb", tag="y")
                            ypss = [
                                psp.tile([128, f1 - f0], F32, name=f"yps{fi}", tag=ytag, bufs=ybufs)
                                for fi, (f0, f1) in enumerate(YCH[:2])
                            ]
                            # c-outer: each oTc stationary reused for 2 f-chunks
                            for c in range(C):
                                for fi, (f0, f1) in enumerate(YCH[:2]):
                                    nc.tensor.matmul(
                                        ypss[fi][:],
                                        oTc_sb[:, c, :],
                                        wp_sb[:, c, f0:f1],
                                        start=(c == 0),
                                        stop=(c == C - 1 and not has_bp),
                                    )
                            for fi, (f0, f1) in enumerate(YCH[:2]):
                                if has_bp:
                                    nc.tensor.matmul(
                                        ypss[fi][:],
                                        ones_sb[0:1, :],
                                        bp_sb[0:1, f0:f1],
                                        start=False,
                                        stop=True,
                                    )
                                # alternate psum-evacuation engines so yps
                                # recycling isn't gated on one engine
                                if fi == 1:
                                    nc.scalar.copy(ysb[:, f0:f1], ypss[fi][:])
                                else:
                                    nc.vector.tensor_copy(ysb[:, f0:f1], ypss[fi][:])
                                nc.sync.dma_start(
                                    out_d[grt * 128 : (grt + 1) * 128, f0:f1], ysb[:, f0:f1]
                                )
                            f0, f1 = YCH[2]
                            yps2 = psp.tile([128, f1 - f0], F32, name="yps2", tag=ytag, bufs=ybufs)
                            for c in range(C):
                                nc.tensor.matmul(
                                    yps2[:],
                                    oTc_sb[:, c, :],
                                    wp_sb[:, c, f0:f1],
                                    start=(c == 0),
                                    stop=(c == C - 1 and not has_bp),
                                )
                            if has_bp:
                                nc.tensor.matmul(
                                    yps2[:],
                                    ones_sb[0:1, :],
                                    bp_sb[0:1, f0:f1],
                                    start=False,
                                    stop=True,
                                )
                            nc.vector.tensor_copy(ysb[:, f0:f1], yps2[:])
                            nc.sync.dma_start(
                                out_d[grt * 128 : (grt + 1) * 128, f0:f1], ysb[:, f0:f1]
                            )
                    for g in range(groups):
                        fillers = q_proj_chunks(g + 1) if g + 1 < groups else []
                        attention(g, fillers)

    nc.compile()
    return nc


_programs = {}


def _get_program(key):
    if key not in _programs:
        _programs[key] = build_program(*key)
    return _programs[key]


def make_in_maps(x, cond, mask, Wq, bq, Wkv, bkv, Wp, bp, rpc=ROWS_PER_CORE, ncores=NCORES):
    """Host-side shard + relayout. Returns (in_maps, flags)."""
    x = np.asarray(x, np.float32)
    cond = np.asarray(cond, np.float32)
    mask = np.asarray(mask)
    Wq = np.asarray(Wq, np.float32)
    Wkv = np.asarray(Wkv, np.float32)
    Wp = np.asarray(Wp, np.float32)
    bq = np.asarray(bq, np.float32)
    bkv = np.asarray(bkv, np.float32)
    bp = np.asarray(bp, np.float32)

    def pmajor(w):  # [D, D2] -> [128, C, D2] partition-major
        d2 = w.shape[1]
        return np.ascontiguousarray(
            w.astype(NPBF16).reshape(C, 128, d2).transpose(1, 0, 2)
        )

    # [C_out, 128, C_in, 128]: wq[c, p, k, q] = Wq[k*128+p, c*128+q]
    wq = np.ascontiguousarray(
        Wq.astype(NPBF16).reshape(C, 128, C, 128).transpose(2, 1, 0, 3)
    )
    wk = np.ascontiguousarray(
        Wkv[:, :D].astype(NPBF16).reshape(C, 128, C, 128).transpose(2, 1, 0, 3)
    )
    wv = pmajor(Wkv[:, D:])
    wp = pmajor(Wp)
    bq_a = np.ascontiguousarray(bq.reshape(C, 128).T)
    bk_a = np.ascontiguousarray(bkv[:D].reshape(C, 128).T)
    bv_a = bkv[D:].astype(NPBF16).reshape(1, D)
    bp_a = bp.astype(NPBF16).reshape(1, D)

    flags = (rpc, bool(bq.any()), bool(bkv[:D].any()), bool(bkv[D:].any()), bool(bp.any()))
    hmask = _hmask_host()

    halves = NSEQ // rpc
    groups = rpc // RG
    in_maps = []
    for core in range(ncores):
        b, half = core // halves, core % halves
        rows = slice(half * rpc, (half + 1) * rpc)
        # [128, G, C, RG]: xT[p, g, c, r] = x[b, g*RG+r, c*128+p]
        xT = np.ascontiguousarray(
            x[b, rows].T.astype(NPBF16).reshape(C, 128, groups, RG).transpose(1, 2, 0, 3)
        )
        condT = np.ascontiguousarray(
            cond[b].T.astype(NPBF16).reshape(C, 128, MKEY).transpose(1, 0, 2)
        )
        mv = (np.arange(MP) < int(mask[b])).astype(np.float32)
        vscale = np.ascontiguousarray(np.exp(mv).reshape(KC, 128).T)
        in_maps.append(
            {
                "xT": xT,
                "condT": condT,
                "wq": wq,
                "wk": wk,
                "wv": wv,
                "wp": wp,
                "bq": bq_a,
                "bk": bk_a,
                "bv": bv_a,
                "bp": bp_a,
                "vscale": vscale,
                "hmask": hmask,
            }
        )
    return in_maps, flags


def kernel(x, cond, mask, Wq, bq, Wkv, bkv, Wp, bp):
    global LAST_EXEC_NS
    import os
    import time

    in_maps, flags = make_in_maps(x, cond, mask, Wq, bq, Wkv, bkv, Wp, bp)
    nc = _get_program(flags)
    trace = bool(os.environ.get("BASS_KERNEL_TRACE"))
    res = None
    for attempt in range(3):
        try:
            res = run_bass_kernel_spmd(nc, in_maps, list(range(NCORES)), trace=trace)
            break
        except Exception:
            if attempt == 2:
                raise
            time.sleep(10)
    LAST_EXEC_NS = res.exec_time_ns
    globals()["LAST_RESULT"] = res

    rpc = flags[0]
    halves = NSEQ // rpc
    out = np.empty((B, NSEQ, D), np.float32)
    for core in range(NCORES):
        b, half = core // halves, core % halves
        out[b, half * rpc : (half + 1) * rpc] = res.results[core]["out"].astype(np.float32)
    return out



# revision 16
# speedup vs baseline: 1.1619x; 1.0715x over previous
"""Multi-head cross-attention Trainium2 kernel (8-core SPMD, data-parallel).

Shards (batch=4) x (seq halves) across 8 NeuronCores; each core runs the
full q/kv/attention/out-proj pipeline for its 2048 query rows in bf16 with
fp32 PSUM accumulation.

Key tricks:
  - mask: reference adds +1.0 to logits of keys j < mask[b] before softmax.
    softmax(l + m) = e^m * e^l / sum  ->  fold e^m into V rows (and into the
    softmax-sum ones column), so masking costs nothing per tile.
  - softmax sums come from an extra ones column appended to V (head_dim 73);
    no vector reductions at all.
  - per-head K^T tiles are zero-padded to full 128-partition chunks so every
    matmul operand sits at base partition 0 (tile_position constraint).
  - normalization (1/sum) is applied after transposing per-head output to
    natural orientation, where rows are partitions and tensor_scalar works.

Round 2 (327us -> target ~280us):
  - PV operand swap: exp is the STATIONARY operand ([keys, row-tile]) and V
    the moving one ([keys, 73]); output lands row-major so normalization is
    a per-partition multiply. Removes the per-head output transposes and the
    oT psum-evacuation cast, and cuts PV moving rows from 1536 to 876 per
    head-group (stationary reloads are hidden: measured 39ns issue spacing
    on 73-row-moving matmuls).
  - q-proj for group g+1 is interleaved one column-chain at a time into
    attention(g)'s head loop: the exp chain on ACT (~1.7us/head) exceeds the
    PE's per-head work (~1.1us), so the PE fills the gap with next-group
    q-proj matmuls instead of idling.

Changes vs the original baseline (381us -> ~319us at full clock):
  - weights/activations stored partition-major in DRAM so each tensor loads
    with a few large DMAs (contiguous per-partition runs) instead of 9 each;
    wq/wk additionally output-chunk-major so the first q-proj / K-proj
    column chains are gated by one 295KB DMA, not the full weight.
  - PSUM tag budgeting: lps+qps "big" (4 bufs), ops/onp/tps "att" (2),
    yps/kps/vps "yk" (2) = 8 banks.
  - two-stage software pipelining in the head loop: logits+exp for head h,
    PV+psum-evacuation for h-1, transpose+normalize for h-2. Removes PE
    head-of-line blocking on the ACT exp chain and on the DVE oT cast.
  - the 4 per-head 1/sum multiplies merged into one tensor_tensor with a
    stride-0 broadcast AP (onat is one [128, 4, D] tile).
  - oTc/ysb copies on DVE (with one ysb chunk on ACT) — out-proj phase was
    ACT-bound; kv K/V chunk emission interleaved.
  - out-proj loops c-outer so each oTc stationary is reused for multiple
    f-chunks; last group's yps tiles borrow the idle "big" slots.
  - output stored bf16 (halves output DMA; host casts back to f32).
  - fp8 was evaluated and REJECTED: host simulation gives rel err 4.5e-2+
    for any fp8 matmul path vs the 2e-2 budget (bf16 pipeline is 7.4e-3).
"""

import sys

sys.path.insert(0, "/opt/trn_rl_repo")

import ml_dtypes
import numpy as np

import concourse.bass as bass  # noqa: F401  (engine types via nc)
import concourse.mybir as mybir
import concourse.tile as tile
from concourse import bacc
from concourse.bass_utils import run_bass_kernel_spmd
from concourse.masks import make_identity

BF16 = mybir.dt.bfloat16
F32 = mybir.dt.float32
NPBF16 = ml_dtypes.bfloat16
AF = mybir.ActivationFunctionType

B, NSEQ, MKEY, D, H, DH = 4, 4096, 300, 1152, 16, 72
NCORES = 8
C = D // 128  # 9 feature chunks
KC = 3  # key chunks, keys padded 300 -> 384
MP = KC * 128
RG = 512  # query rows per group
SCALE = 1.0 / float(np.sqrt(DH))
ROWS_PER_CORE = B * NSEQ // NCORES  # 2048

LAST_EXEC_NS = None
LAST_RESULT = None


def _head_segs(h):
    """Feature range [72h, 72h+72) of head h split at 128-chunk boundaries.

    Returns [(chunk, lo, hi)] with chunk-local partition range [lo, hi)."""
    f0, f1 = DH * h, DH * h + DH
    segs = []
    c = f0 // 128
    while c * 128 < f1:
        lo = max(f0, c * 128) - c * 128
        hi = min(f1, (c + 1) * 128) - c * 128
        segs.append((c, lo, hi))
        c += 1
    return segs


def _chunk_segs(c):
    """[(h, i, lo, hi)] head segments living in feature chunk c."""
    out = []
    for h in range(H):
        for i, (hc, lo, hi) in enumerate(_head_segs(h)):
            if hc == c:
                out.append((h, i, lo, hi))
    return out


# flat order of all (head, segment) pairs; column index into the hmask input
_ALL_SEGS = [(h, i) for h in range(H) for i in range(len(_head_segs(h)))]
_SEG_IDX = {hs: s for s, hs in enumerate(_ALL_SEGS)}
NSEG = len(_ALL_SEGS)


def _hmask_host():
    """[128, NSEG] f32: column (h,i) is 1.0 on the chunk-local partitions of
    that head segment, 0 elsewhere. Engine ops can't address SBUF at
    non-32-aligned partition bases, so head extraction is done as a
    full-chunk copy multiplied by this per-partition mask."""
    m = np.zeros((128, NSEG), np.float32)
    for h in range(H):
        for i, (_, lo, hi) in enumerate(_head_segs(h)):
            m[lo:hi, _SEG_IDX[(h, i)]] = 1.0
    return m


def build_program(rpc=ROWS_PER_CORE, has_bq=False, has_bk=False, has_bv=False, has_bp=False):
    nc = bacc.Bacc()

    groups = rpc // RG
    tiles_per_group = RG // 128
    kn = [128, 128, MKEY - 256]  # real keys per key chunk

    # partition-major layouts: one big DMA per tensor
    xT_d = nc.dram_tensor("xT", [128, groups, C, RG], BF16, kind="ExternalInput")
    condT_d = nc.dram_tensor("condT", [128, C, MKEY], BF16, kind="ExternalInput")
    # wq is output-chunk-major: wq[c][p][k][q] = Wq[k*128+p, c*128+q], so the
    # first q-proj column chain only waits for one 295KB DMA (plus xT)
    wq_d = nc.dram_tensor("wq", [C, 128, C, 128], BF16, kind="ExternalInput")
    wk_d = nc.dram_tensor("wk", [C, 128, C, 128], BF16, kind="ExternalInput")
    wv_d = nc.dram_tensor("wv", [128, C, D], BF16, kind="ExternalInput")
    wp_d = nc.dram_tensor("wp", [128, C, D], BF16, kind="ExternalInput")
    bq_d = nc.dram_tensor("bq", [128, C], F32, kind="ExternalInput")
    bk_d = nc.dram_tensor("bk", [128, C], F32, kind="ExternalInput")
    bv_d = nc.dram_tensor("bv", [1, D], BF16, kind="ExternalInput")
    bp_d = nc.dram_tensor("bp", [1, D], BF16, kind="ExternalInput")
    vs_d = nc.dram_tensor("vscale", [128, KC], F32, kind="ExternalInput")
    hm_d = nc.dram_tensor("hmask", [128, NSEG], F32, kind="ExternalInput")
    out_d = nc.dram_tensor("out", [rpc, D], BF16, kind="ExternalOutput")

    YCH = [(0, 384), (384, 768), (768, 1152)]

    with tile.TileContext(nc) as tc:
        with (
            tc.tile_pool(name="const", bufs=1) as cpool,
            tc.tile_pool(name="ps", bufs=3, space="PSUM") as psp,
        ):
            # persistent weights / constants (split big loads in thirds so the
            # first q-proj matmuls can start after ~1/3 of the bytes land).
            # wq/wk are out-chunk-major in SBUF so each chunk DMA writes one
            # contiguous 2304B run per partition instead of 9x 256B runs
            # (~2x effective DMA bandwidth on the startup-critical bytes).
            wq_sb = cpool.tile([128, C, C, 128], BF16)
            wp_sb = cpool.tile([128, C, D], BF16)
            ident = cpool.tile([128, 128], BF16)
            make_identity(nc, ident[:])
            # vs/hm are tiny and first needed in the kv phase; issue their
            # DMAs after the startup-critical wq/xT loads (see q_proj)
            vs_sb = cpool.tile([128, KC], F32)
            hm_sb = cpool.tile([128, NSEG], F32)
            if has_bq:
                bq_sb = cpool.tile([128, C], F32)
                nc.sync.dma_start(bq_sb[:], bq_d[:])
            if has_bk:
                bk_sb = cpool.tile([128, C], F32)
                nc.sync.dma_start(bk_sb[:], bk_d[:])
            if has_bp:
                bp_sb = cpool.tile([1, D], BF16)
                nc.sync.dma_start(bp_sb[:], bp_d[:])
            if has_bv or has_bp:
                ones_sb = cpool.tile([1, 128], BF16)
                nc.gpsimd.memset(ones_sb[:], 1.0)

            # V in natural orientation [key, head, dim+1]; fake keys stay 0,
            # col 72 holds e^mask (ones column pre-scaled by the mask factor)
            v_sb = cpool.tile([128, KC, H, DH + 1], BF16)
            nc.gpsimd.memset(v_sb[:], 0.0)
            kTz = {}
            for h in range(H):
                for i in range(len(_head_segs(h))):
                    t = cpool.tile([128, MP], BF16, name=f"kTz_{h}_{i}")
                    nc.gpsimd.memset(t[:], 0.0)
                    kTz[(h, i)] = t

            kT72 = {}
            for h in range(H):
                if len(_head_segs(h)) == 2:
                    t = cpool.tile([DH + 4, MP], BF16, name=f"kT72_{h}")
                    nc.gpsimd.memset(t[:], 0.0)
                    kT72[h] = t

            # ---- streaming: q-proj, attention, out-proj ----
            with tc.tile_pool(name="xq", bufs=2) as xqpool:
                qts = {}
                xqt = {}
                first = [True]

                def q_dma(g):
                    """Issue the xT DMA for group g and allocate its qT tile.
                    Called well ahead of the matmuls (during the previous
                    group's out-proj) so the 1.2MB xT load is never on the
                    critical path of the interleaved q-proj chains."""
                    xT_sb = xqpool.tile([128, C, RG], BF16, name="xT", tag="xT")
                    if first[0]:
                        # startup: first q-proj output chunk is gated only by
                        # wq col-chunk 0 and xT; remaining wq chunks stream in
                        nc.sync.dma_start(wq_sb[:, 0], wq_d[0])
                        for k0, k1 in [(0, 3), (3, 6), (6, C)]:
                            nc.sync.dma_start(xT_sb[:, k0:k1, :], xT_d[:, g, k0:k1, :])
                        for c in range(1, C):
                            nc.sync.dma_start(wq_sb[:, c], wq_d[c])
                        nc.sync.dma_start(vs_sb[:], vs_d[:])
                        nc.sync.dma_start(hm_sb[:], hm_d[:])
                        first[0] = False
                    else:
                        nc.sync.dma_start(xT_sb[:], xT_d[:, g])
                    qT_sb = xqpool.tile([128, C, RG], BF16, name="qT", tag="qT")
                    qts[g] = qT_sb
                    xqt[g] = xT_sb

                def q_proj_chunks(g):
                    """One thunk per q-proj output chunk (9 matmuls + psum
                    evacuation each); q_dma(g) must have been called."""
                    xT_sb = xqt.pop(g)
                    qT_sb = qts[g]

                    def chunk(c):
                        qps = psp.tile([128, RG], F32, name="qps", tag="big", bufs=4)
                        for k in range(C):
                            nc.tensor.matmul(
                                qps[:],
                                wq_sb[:, c, k, :],
                                xT_sb[:, k, :],
                                start=(k == 0),
                                stop=(k == C - 1),
                            )
                        if has_bq:
                            nc.scalar.activation(
                                qT_sb[:, c, :], qps[:], AF.Identity, bias=bq_sb[:, c : c + 1]
                            )
                        else:
                            nc.vector.tensor_copy(qT_sb[:, c, :], qps[:])

                    return [lambda c=c: chunk(c) for c in range(C)]

                def q_proj(g):
                    for f in q_proj_chunks(g):
                        f()

                q_dma(0)
                q_proj(0)
                # ---- kv projection (weights in a scoped SBUF pool) ----
                with tc.tile_pool(name="kvw", bufs=1) as kvpool:
                    condT_sb = kvpool.tile([128, C, MKEY], BF16)
                    wk_sb = kvpool.tile([128, C, C, 128], BF16)
                    wv_sb = kvpool.tile([128, C, D], BF16)
                    nc.sync.dma_start(condT_sb[:], condT_d[:])
                    # wk by output chunk (contiguous 2304B runs); wv in two
                    # halves (1440B/864B runs) so the V chains start early
                    for c in range(C):
                        nc.sync.dma_start(wk_sb[:, c], wk_d[c])
                        if c == 0:
                            nc.sync.dma_start(wv_sb[:, :, 0:720], wv_d[:, :, 0:720])
                        elif c == 1:
                            nc.sync.dma_start(wv_sb[:, :, 720:D], wv_d[:, :, 720:D])
                    # group 1's activations land during the kv phase, ahead of
                    # wp (first needed ~20us later in out-proj(0))
                    q_dma(1)
                    nc.sync.dma_start(wp_sb[:], wp_d[:])
                    if has_bv:
                        bv_sb = kvpool.tile([1, D], BF16)
                        nc.sync.dma_start(bv_sb[:], bv_d[:])

                    # K^T in feature-chunk orientation -> zero-padded head tiles
                    def emit_k(c):
                        kps = psp.tile([128, MKEY], F32, name="kps", tag="yk", bufs=2)
                        for k in range(C):
                            nc.tensor.matmul(
                                kps[:],
                                wk_sb[:, c, k, :],
                                condT_sb[:, k, :],
                                start=(k == 0),
                                stop=(k == C - 1),
                            )
                        for h, i, _lo, _hi in _chunk_segs(c):
                            s = _SEG_IDX[(h, i)]
                            if has_bk:
                                nc.vector.tensor_scalar(
                                    kTz[(h, i)][:, 0:MKEY],
                                    kps[:],
                                    bk_sb[:, c : c + 1],
                                    hm_sb[:, s : s + 1],
                                    op0=mybir.AluOpType.add,
                                    op1=mybir.AluOpType.mult,
                                )
                            else:
                                nc.vector.tensor_scalar_mul(
                                    kTz[(h, i)][:, 0:MKEY], kps[:], hm_sb[:, s : s + 1]
                                )

                    # V natural [keys, feat], head-aligned 360-wide chunks
                    vch = [(0, 360), (360, 720), (720, 1080), (1080, 1152)]

                    def emit_v(kc, f0, f1):
                        vps = psp.tile([128, f1 - f0], F32, name="vps", tag="yk", bufs=2)
                        for k in range(C):
                            nc.tensor.matmul(
                                vps[0 : kn[kc], :],
                                condT_sb[:, k, kc * 128 : kc * 128 + kn[kc]],
                                wv_sb[:, k, f0:f1],
                                start=(k == 0),
                                stop=(k == C - 1 and not has_bv),
                            )
                        if has_bv:
                            nc.tensor.matmul(
                                vps[0 : kn[kc], :],
                                ones_sb[0:1, 0 : kn[kc]],
                                bv_sb[0:1, f0:f1],
                                start=False,
                                stop=True,
                            )
                        for h in range(f0 // DH, f1 // DH):
                            d0 = h * DH - f0
                            nc.vector.tensor_scalar_mul(
                                v_sb[0 : kn[kc], kc, h, 0:DH],
                                vps[0 : kn[kc], d0 : d0 + DH],
                                vs_sb[0 : kn[kc], kc : kc + 1],
                            )

                    # interleave K and V chunks: the PE streams one chain
                    # while the DVE drains the other's extraction muls
                    vlist = [(kc, f0, f1) for kc in range(KC) for (f0, f1) in vch]
                    for j in range(max(C, len(vlist))):
                        if j < C:
                            emit_k(j)
                        if j < len(vlist):
                            emit_v(*vlist[j])
                    for kc in range(KC):
                        for h in range(H):
                            nc.any.tensor_copy(
                                v_sb[0 : kn[kc], kc, h, DH : DH + 1],
                                vs_sb[0 : kn[kc], kc : kc + 1],
                            )

                    for h, t in kT72.items():
                        (c0, lo0, hi0), (c1, lo1, hi1) = _head_segs(h)
                        n0 = hi0 - lo0
                        nc.sync.dma_start(t[0:n0, 0:MKEY], kTz[(h, 0)][lo0:hi0, 0:MKEY])
                        nc.sync.dma_start(t[n0 : n0 + (hi1 - lo1), 0:MKEY], kTz[(h, 1)][lo1:hi1, 0:MKEY])
                with (
                    tc.tile_pool(name="att", bufs=4) as apool,
                    tc.tile_pool(name="outp", bufs=2) as opool,
                ):
                    qTgs = {}
                    hoisted = {}

                    def make_qTg(g):
                        qT_sb = qts[g]
                        qTg = {}
                        for h in sorted(kT72):
                            (c0, lo0, hi0), (c1, lo1, hi1) = _head_segs(h)
                            n0 = hi0 - lo0
                            t = opool.tile([DH + 4, RG], BF16, name=f"qTg{h}", tag=f"qTg{h}")
                            nc.sync.dma_start(t[0:n0, :], qT_sb[lo0:hi0, c0, :])
                            nc.sync.dma_start(t[n0:DH, :], qT_sb[lo1:hi1, c1, :])
                            qTg[h] = t
                        qTgs[g] = qTg

                    def logits_exp(g, h):
                        """3 logits matmuls + 3 exps for head h of group g;
                        returns the expT tile (PV stationary)."""
                        expT = apool.tile([128, KC, RG], BF16, name="expT", tag="expT", bufs=12)
                        for kc in range(KC):
                            lps = psp.tile([128, RG], F32, name="lps", tag="big", bufs=4)
                            if h in kT72:
                                nc.tensor.matmul(
                                    lps[:],
                                    kT72[h][0:DH, kc * 128 : (kc + 1) * 128],
                                    qTgs[g][h][0:DH, :],
                                    start=True,
                                    stop=True,
                                )
                            else:
                                (c, lo, hi) = _head_segs(h)[0]
                                nc.tensor.matmul(
                                    lps[:],
                                    kTz[(h, 0)][:, kc * 128 : (kc + 1) * 128],
                                    qts[g][:, c, :],
                                    start=True,
                                    stop=True,
                                )
                            nc.scalar.activation(expT[:, kc, :], lps[:], AF.Exp, scale=SCALE)
                        return expT

                    def attention(g, fillers=()):
                        # the last group's out-proj has no successor work to
                        # hide psum-evacuation waits; borrow the (idle) lps/qps
                        # slots for deeper yps pipelining there
                        ytag, ybufs = ("big", 4) if g == groups - 1 else ("yk", 2)
                        fillers = list(fillers)
                        if g not in qTgs:
                            make_qTg(g)

                        # one [128, rt, feat] natural-orientation tile per group
                        onat = opool.tile(
                            [128, tiles_per_group, D], BF16, name="onat", tag="onat"
                        )

                        def head_pv(h, expT):
                            """PV for head h with exp as the STATIONARY operand
                            (one head behind the logits/exp front).

                            out[row, dim] = sum_k exp[k, row] * V[k, dim] comes
                            out row-major, so softmax normalization is a plain
                            per-partition multiply and no transposes are needed.
                            12 matmuls of 73 moving rows (876 cycles) replace
                            3 of 512 (1536); the measured issue rate of short-
                            moving matmuls shows stationary reloads are hidden.
                            """
                            ops = psp.tile(
                                [128, tiles_per_group, DH + 4], F32, name="ops", tag="att", bufs=2
                            )
                            # rt-OUTER so each accumulation group (kc chain) is
                            # contiguous: start=True clears has_written bits for
                            # the WHOLE bank, so interleaving groups within one
                            # bank corrupts earlier partial sums. A finished
                            # group's values survive later bit-clears (only the
                            # bits reset, not the data).
                            for rt in range(tiles_per_group):
                                for kc in range(KC):
                                    nc.tensor.matmul(
                                        ops[:, rt, 0 : DH + 1],
                                        expT[:, kc, rt * 128 : (rt + 1) * 128],
                                        v_sb[:, kc, h, :],
                                        start=(kc == 0),
                                        stop=(kc == KC - 1),
                                    )
                            inv = apool.tile([128, tiles_per_group], F32, name="inv", tag="inv")
                            nc.vector.reciprocal(inv[:], ops[:, :, DH])
                            # all 4 row-tiles normalized in one op (stride-0
                            # broadcast of inv along the feature dim)
                            nc.vector.tensor_mul(
                                onat[:, :, h * DH : (h + 1) * DH],
                                ops[:, :, 0:DH],
                                inv[:].unsqueeze(2).broadcast_to([128, tiles_per_group, DH]),
                            )

                        # pv_queue holds heads whose exp is issued but PV isn't.
                        # Hoisted heads (logits+exp already run during the
                        # previous group's out-proj) seed the queue; the loop
                        # then only issues logits+exp for the remaining heads,
                        # draining npop PVs per slot.
                        head_order = [h for h in range(H) if h not in kT72] + sorted(kT72)
                        seeded = sorted(h for (gg, h) in hoisted if gg == g)
                        pv_queue = [(h, hoisted.pop((g, h))) for h in seeded]
                        loop_heads = [h for h in head_order if h not in seeded]
                        npop = 2 if pv_queue else 1
                        for hi, h in enumerate(loop_heads):
                            expT = logits_exp(g, h)
                            for _ in range(npop):
                                if len(pv_queue) > (0 if npop == 2 else 1):
                                    head_pv(*pv_queue.pop(0))
                            # ACT (3 exps, ~1.7us) outruns the PE's own work in
                            # a head slot (~1.1us); interleave one next-group
                            # q-proj column chain every other head so the PE
                            # stays busy instead of idling on the exp chain
                            if hi % 2 == 0 and fillers:
                                fillers.pop(0)()
                            pv_queue.append((h, expT))
                        while pv_queue:
                            head_pv(*pv_queue.pop(0))
                        while fillers:
                            fillers.pop(0)()

                        # prefetch: group g+2's xT DMA rides under this
                        # out-proj; g+1's last-group hoist (see below) needs
                        # its qTg assembled ahead of time as well
                        if g + 2 < groups:
                            q_dma(g + 2)
                        hoist = []
                        if g == groups - 2:
                            # the last group has no q-proj filler, so its head
                            # loop would serialize on the 16-exp ACT chain.
                            # Run the 8 single-segment heads' logits+exp here
                            # (ACT is nearly idle during out-proj) so
                            # attention(groups-1) only has 8 exps left to hide.
                            make_qTg(groups - 1)
                            gl = groups - 1
                            hoist = [
                                (lambda h=h: hoisted.__setitem__((gl, h), logits_exp(gl, h)))
                                for h in range(H)
                                if h not in kT72
                            ]
                        for rt in range(tiles_per_group):
                            grt = g * tiles_per_group + rt
                            oTc_sb = opool.tile([128, C, 128], BF16, name="oTc", tag="oTc", bufs=3)
                            for c3 in range(C // 3):
                                tps = psp.tile([128, 3, 128], BF16, name="tps", tag="att", bufs=2)
                                for j in range(3):
                                    c = c3 * 3 + j
                                    nc.tensor.transpose(
                                        tps[:, j, :],
                                        onat[:, rt, c * 128 : (c + 1) * 128],
                                        ident[:],
                                    )
                                nc.vector.tensor_copy(oTc_sb[:, c3 * 3 : c3 * 3 + 3, :], tps[:])

                            ysb = opool.tile([128, D], BF16, name="ysb", tag="y", bufs=4)
                            ypss = [
                                psp.tile([128, f1 - f0], F32, name=f"yps{fi}", tag=ytag, bufs=ybufs)
                                for fi, (f0, f1) in enumerate(YCH[:2])
                            ]
                            # c-outer: each oTc stationary reused for 2 f-chunks
                            for c in range(C):
                                for fi, (f0, f1) in enumerate(YCH[:2]):
                                    nc.tensor.matmul(
                                        ypss[fi][:],
                                        oTc_sb[:, c, :],
                                        wp_sb[:, c, f0:f1],
                                        start=(c == 0),
                                        stop=(c == C - 1 and not has_bp),
                                    )
                            for fi, (f0, f1) in enumerate(YCH[:2]):
                                if has_bp:
                                    nc.tensor.matmul(
                                        ypss[fi][:],
                                        ones_sb[0:1, :],
                                        bp_sb[0:1, f0:f1],
                                        start=False,
                                        stop=True,
                                    )
                                # alternate psum-evacuation engines so yps
                                # recycling isn't gated on one engine
                                if fi == 1:
                                    nc.scalar.copy(ysb[:, f0:f1], ypss[fi][:])
                                else:
                                    nc.vector.tensor_copy(ysb[:, f0:f1], ypss[fi][:])
                                nc.sync.dma_start(
                                    out_d[grt * 128 : (grt + 1) * 128, f0:f1], ysb[:, f0:f1]
                                )
                            f0, f1 = YCH[2]
                            yps2 = psp.tile([128, f1 - f0], F32, name="yps2", tag=ytag, bufs=ybufs)
                            for c in range(C):
                                nc.tensor.matmul(
                                    yps2[:],
                                    oTc_sb[:, c, :],
                                    wp_sb[:, c, f0:f1],
                                    start=(c == 0),
                                    stop=(c == C - 1 and not has_bp),
                                )
                            if has_bp:
                                nc.tensor.matmul(
                                    yps2[:],
                                    ones_sb[0:1, :],
                                    bp_sb[0:1, f0:f1],
                                    start=False,
                                    stop=True,
                                )
                            nc.vector.tensor_copy(ysb[:, f0:f1], yps2[:])
                            nc.sync.dma_start(
                                out_d[grt * 128 : (grt + 1) * 128, f0:f1], ysb[:, f0:f1]
                            )
                            # two hoisted last-group heads per row-tile
                            for _ in range(2):
                                if hoist:
                                    hoist.pop(0)()
                        while hoist:
                            hoist.pop(0)()
                    for g in range(groups):
                        fillers = q_proj_chunks(g + 1) if g + 1 < groups else []
                        attention(g, fillers)

    nc.compile()
    return nc


_programs = {}


def _get_program(key):
    if key not in _programs:
        _programs[key] = build_program(*key)
    return _programs[key]


def make_in_maps(x, cond, mask, Wq, bq, Wkv, bkv, Wp, bp, rpc=ROWS_PER_CORE, ncores=NCORES):
    """Host-side shard + relayout. Returns (in_maps, flags)."""
    x = np.asarray(x, np.float32)
    cond = np.asarray(cond, np.float32)
    mask = np.asarray(mask)
    Wq = np.asarray(Wq, np.float32)
    Wkv = np.asarray(Wkv, np.float32)
    Wp = np.asarray(Wp, np.float32)
    bq = np.asarray(bq, np.float32)
    bkv = np.asarray(bkv, np.float32)
    bp = np.asarray(bp, np.float32)

    def pmajor(w):  # [D, D2] -> [128, C, D2] partition-major
        d2 = w.shape[1]
        return np.ascontiguousarray(
            w.astype(NPBF16).reshape(C, 128, d2).transpose(1, 0, 2)
        )

    # [C_out, 128, C_in, 128]: wq[c, p, k, q] = Wq[k*128+p, c*128+q]
    wq = np.ascontiguousarray(
        Wq.astype(NPBF16).reshape(C, 128, C, 128).transpose(2, 1, 0, 3)
    )
    wk = np.ascontiguousarray(
        Wkv[:, :D].astype(NPBF16).reshape(C, 128, C, 128).transpose(2, 1, 0, 3)
    )
    wv = pmajor(Wkv[:, D:])
    wp = pmajor(Wp)
    bq_a = np.ascontiguousarray(bq.reshape(C, 128).T)
    bk_a = np.ascontiguousarray(bkv[:D].reshape(C, 128).T)
    bv_a = bkv[D:].astype(NPBF16).reshape(1, D)
    bp_a = bp.astype(NPBF16).reshape(1, D)

    flags = (rpc, bool(bq.any()), bool(bkv[:D].any()), bool(bkv[D:].any()), bool(bp.any()))
    hmask = _hmask_host()

    halves = NSEQ // rpc
    groups = rpc // RG
    in_maps = []
    for core in range(ncores):
        b, half = core // halves, core % halves
        rows = slice(half * rpc, (half + 1) * rpc)
        # [128, G, C, RG]: xT[p, g, c, r] = x[b, g*RG+r, c*128+p]
        xT = np.ascontiguousarray(
            x[b, rows].T.astype(NPBF16).reshape(C, 128, groups, RG).transpose(1, 2, 0, 3)
        )
        condT = np.ascontiguousarray(
            cond[b].T.astype(NPBF16).reshape(C, 128, MKEY).transpose(1, 0, 2)
        )
        mv = (np.arange(MP) < int(mask[b])).astype(np.float32)
        vscale = np.ascontiguousarray(np.exp(mv).reshape(KC, 128).T)
        in_maps.append(
            {
                "xT": xT,
                "condT": condT,
                "wq": wq,
                "wk": wk,
                "wv": wv,
                "wp": wp,
                "bq": bq_a,
                "bk": bk_a,
                "bv": bv_a,
                "bp": bp_a,
                "vscale": vscale,
                "hmask": hmask,
            }
        )
    return in_maps, flags


def kernel(x, cond, mask, Wq, bq, Wkv, bkv, Wp, bp):
    global LAST_EXEC_NS
    import os
    import time

    in_maps, flags = make_in_maps(x, cond, mask, Wq, bq, Wkv, bkv, Wp, bp)
    nc = _get_program(flags)
    trace = bool(os.environ.get("BASS_KERNEL_TRACE"))
    res = None
    for attempt in range(3):
        try:
            res = run_bass_kernel_spmd(nc, in_maps, list(range(NCORES)), trace=trace)
            break
        except Exception:
            if attempt == 2:
                raise
            time.sleep(10)
    LAST_EXEC_NS = res.exec_time_ns
    globals()["LAST_RESULT"] = res

    rpc = flags[0]
    halves = NSEQ // rpc
    out = np.empty((B, NSEQ, D), np.float32)
    for core in range(NCORES):
        b, half = core // halves, core % halves
        out[b, half * rpc : (half + 1) * rpc] = res.results[core]["out"].astype(np.float32)
    return out

